# revision 1
# baseline (speedup 1.0000x reference)
"""Trainium2 Bass kernel for nn_HarmonicOscillatorOrbitals.

out[b, i, j] = exp(-s^2/2) * H_j(s), s = omega * x[b, i, 0], j = 0..31
(physicists' Hermite polynomials), data-parallel over 8 NeuronCores on
the leading batch axis.

Per core (8192 batches = 262144 scalars as [128 partitions, E=2048]):
  t   = 2*omega*x
  env = exp(-s^2/2) = 2^(t^2 * -log2(e)/8), computed exactly on DVE:
        2^n by float-magic + integer exponent shift, 2^f by a degree-5
        polynomial (fused scalar_tensor_tensor Horner chain) — the ACT
        spline Exp is ~1e-5 off, this path is ~1e-6.
  G_0 = env, G_1 = t*env, G_k = t*G_{k-1} - 2(k-1)*G_{k-2}  (= env*H_k)

The per-element recurrence is serial in k, so elements are split into
two independent column slices: DVE runs one chain (tensor_mul + fused
scalar_tensor_tensor), GPSIMD the other (tensor_mul + tensor_sub, with
ACT supplying the 2(k-1)*G_{k-2} scale-copies two steps ahead). Each
chain keeps its engine fully busy with no cross-engine ping-pong.

G_k slices stay contiguous in SBUF; DRAM output is k-major
[128, 32, E] (1.3KB DMA descriptors) and the host permutes to
(batch, i, j) while unsharding.
"""

from contextlib import ExitStack

import numpy as np

import concourse.bacc as bacc
import concourse.mybir as mybir
import concourse.tile as tile
from concourse.bass_utils import run_bass_kernel_spmd

F32 = mybir.dt.float32
I32 = mybir.dt.int32
AF = mybir.ActivationFunctionType
ALU = mybir.AluOpType

NJ = 32          # number of Hermite orders
N_CORES = 8
B = 65536        # full batch
BC = B // N_CORES
E = BC * NJ // 128   # 2048 free elems per partition per core

TILE_F = 512     # columns per tile
FD = 300         # DVE-owned columns per tile (rest on GPSIMD)

# exp2: env = 2^v, v = sq * K4 with sq = t^2 = 4 s^2
K4 = float(np.float32(-np.log2(np.e) / 8.0))
MAGIC = float(np.float32(1.5 * 2**23))
EXP_B4, EXP_B3, EXP_B2, EXP_B1 = 7.292242, 41.85769, 181.15059, 522.6992
EXP_A5, EXP_A0 = 0.0013260915, 1.0
EXP_EIMM = 127 - 0x4B400000  # (bits(w) + EXP_EIMM) << 23 == bits(2^n)


def _build(e=E, tile_f=TILE_F, fd=FD, accurate_env=False):
    nc = bacc.Bacc("TRN2", target_bir_lowering=False, debug=False)
    x_d = nc.dram_tensor("x", [128, e], F32, kind="ExternalInput").ap()
    om_d = nc.dram_tensor("om", [1, 1], F32, kind="ExternalInput").ap()
    # raw tile dump: per f-tile, the DVE-slice tile [128, NJ*fd] then the
    # GPSIMD-slice tile [128, NJ*fg], verbatim — host unscrambles
    out_d = nc.dram_tensor("out", [128, NJ * e], F32, kind="ExternalOutput").ap()

    fg = tile_f - fd
    n_tiles = e // tile_f
    with tile.TileContext(nc) as tc, ExitStack() as ctx:
        cpool = ctx.enter_context(tc.tile_pool(name="const", bufs=1))
        xp = ctx.enter_context(tc.tile_pool(name="xp", bufs=4))
        ep = ctx.enter_context(tc.tile_pool(name="ep", bufs=3))
        qd = ctx.enter_context(tc.tile_pool(name="qd", bufs=4))
        qg = ctx.enter_context(tc.tile_pool(name="qg", bufs=4))
        gdp = ctx.enter_context(tc.tile_pool(name="gdp", bufs=2))
        ggp = ctx.enter_context(tc.tile_pool(name="ggp", bufs=2))

        om1 = cpool.tile([128, 1], F32)
        nc.sync.dma_start(om1[0:1, :], om_d[:, :])
        om2 = cpool.tile([128, 1], F32)
        nc.gpsimd.partition_broadcast(om2[:, :], om1[0:1, :])
        nc.scalar.mul(om2[:, :], om2[:, :], 2.0)  # om2 = 2*omega

        # int32 constants for the exponent fixup: (bits(w) + EXP_EIMM) << 23
        addc = cpool.tile([128, tile_f], I32)
        nc.vector.memset(addc[:, :], EXP_EIMM)
        t23 = cpool.tile([128, tile_f], I32)
        nc.vector.memset(t23[:, :], 23)

        for it in range(n_tiles):
            f0 = it * tile_f
            x_t = xp.tile([128, tile_f], F32)
            nc.sync.dma_start(x_t[:, :], x_d[:, f0 : f0 + tile_f])
            t_t = xp.tile([128, tile_f], F32, tag="t")
            nc.scalar.mul(t_t[:, :], x_t[:, :], om2[:, 0:1])  # t = 2*omega*x

            # four k-quarters per slice: DMA each out as soon as its 8
            # columns are done, so pool slots recycle at 1/4-tile grain
            gd_q = [
                gdp.tile([128, 8 * fd], F32, name=f"gdq{q}_{it}", tag=f"gd{q}")
                for q in range(4)
            ]
            gg_q = [
                ggp.tile([128, 8 * fg], F32, name=f"ggq{q}_{it}", tag=f"gg{q}")
                for q in range(4)
            ]

            def gds(k):
                return gd_q[k // 8][:, (k % 8) * fd : (k % 8 + 1) * fd]

            def ggs(k):
                return gg_q[k // 8][:, (k % 8) * fg : (k % 8 + 1) * fg]

            base = it * NJ * tile_f

            def flush_quarter(q):
                nc.sync.dma_start(
                    out_d[:, base + q * 8 * fd : base + (q + 1) * 8 * fd],
                    gd_q[q][:, :],
                )
                goff = base + NJ * fd
                nc.sync.dma_start(
                    out_d[:, goff + q * 8 * fg : goff + (q + 1) * 8 * fg],
                    gg_q[q][:, :],
                )

            if accurate_env:
                # ---- exact exp2 on DVE, full tile width ----
                sq = ep.tile([128, tile_f], F32, tag="sq")
                nc.vector.tensor_mul(sq[:, :], t_t[:, :], t_t[:, :])
                v_t = ep.tile([128, tile_f], F32, tag="v")
                nc.vector.tensor_scalar_mul(v_t[:, :], sq[:, :], K4)
                w_t = ep.tile([128, tile_f], F32, tag="w")
                nc.vector.tensor_scalar_add(w_t[:, :], v_t[:, :], MAGIC)
                n_t = ep.tile([128, tile_f], F32, tag="n")
                nc.vector.tensor_scalar_sub(n_t[:, :], w_t[:, :], MAGIC)
                f_t = ep.tile([128, tile_f], F32, tag="f")
                nc.vector.tensor_sub(f_t[:, :], v_t[:, :], n_t[:, :])
                p_t = ep.tile([128, tile_f], F32, tag="p")
                nc.vector.scalar_tensor_tensor(
                    p_t[:, :], f_t[:, :], EXP_B4, f_t[:, :], ALU.add, ALU.mult
                )
                for bb in (EXP_B3, EXP_B2, EXP_B1):
                    nc.vector.scalar_tensor_tensor(
                        p_t[:, :], p_t[:, :], bb, f_t[:, :], ALU.add, ALU.mult
                    )
                nc.vector.tensor_scalar(
                    p_t[:, :], p_t[:, :], EXP_A5, EXP_A0, ALU.mult, ALU.add
                )
                e2_t = ep.tile([128, tile_f], I32, tag="e2")
                nc.vector.tensor_tensor(
                    e2_t[:, :], w_t[:, :].bitcast(I32), addc[:, :], ALU.add
                )
                nc.vector.tensor_tensor(
                    e2_t[:, :], e2_t[:, :], t23[:, :], ALU.logical_shift_left
                )
                e2f = e2_t[:, :].bitcast(F32)
                # env split straight into the two G tiles (k = 0)
                nc.vector.tensor_mul(gds(0), p_t[:, 0:fd], e2f[:, 0:fd])
                nc.vector.tensor_mul(ggs(0), p_t[:, fd:], e2f[:, fd:])
            else:
                sq = ep.tile([128, tile_f], F32, tag="sq")
                nc.scalar.activation(sq[:, :], t_t[:, :], AF.Square, scale=0.5)
                nc.scalar.activation(gds(0), sq[:, 0:fd], AF.Exp, scale=-0.5)
                nc.scalar.activation(ggs(0), sq[:, fd:], AF.Exp, scale=-0.5)

            # G_1 = t * env, each engine seeds its own chain
            nc.vector.tensor_mul(gds(1), t_t[:, 0:fd], gds(0))
            nc.gpsimd.tensor_mul(ggs(1), t_t[:, fd:], ggs(0))

            for k in range(2, NJ):
                c = 2.0 * (k - 1)
                # DVE chain
                q_t = qd.tile([128, fd], F32)
                nc.vector.tensor_mul(q_t[:, :], t_t[:, 0:fd], gds(k - 1))
                nc.vector.scalar_tensor_tensor(
                    gds(k), gds(k - 2), -c, q_t[:, :], ALU.mult, ALU.add
                )
                # GPSIMD chain (ACT supplies c*G_{k-2})
                qg_t = qg.tile([128, fg], F32)
                rg_t = qg.tile([128, fg], F32, tag="rg")
                nc.gpsimd.tensor_mul(qg_t[:, :], t_t[:, fd:], ggs(k - 1))
                nc.scalar.mul(rg_t[:, :], ggs(k - 2), c)
                nc.gpsimd.tensor_sub(ggs(k), qg_t[:, :], rg_t[:, :])
                if k % 8 == 7:
                    flush_quarter(k // 8)

    nc.compile()
    return nc


_CACHED_NC = None


def kernel(x: np.ndarray, omega_kernel: np.ndarray, **run_kwargs) -> np.ndarray:
    global _CACHED_NC
    assert x.shape == (B, NJ, 1) and omega_kernel.shape == (1, 1), (
        x.shape,
        omega_kernel.shape,
    )
    x = np.ascontiguousarray(x, np.float32)
    om = np.ascontiguousarray(omega_kernel, np.float32)

    if _CACHED_NC is None:
        _CACHED_NC = _build()
    nc = _CACHED_NC

    in_maps = [
        {
            "x": x[c * BC : (c + 1) * BC].reshape(128, E),
            "om": om,
        }
        for c in range(N_CORES)
    ]
    res = run_bass_kernel_spmd(nc, in_maps, core_ids=list(range(N_CORES)), **run_kwargs)
    fg = TILE_F - FD
    full = np.empty((B, NJ, NJ), np.float32)
    for c in range(N_CORES):
        arr = np.asarray(res.results[c]["out"]).reshape(128, NJ * E)
        out3 = np.empty((128, NJ, E), np.float32)
        for it in range(E // TILE_F):
            f0, base = it * TILE_F, it * NJ * TILE_F
            out3[:, :, f0 : f0 + FD] = arr[
                :, base : base + NJ * FD
            ].reshape(128, NJ, FD)
            out3[:, :, f0 + FD : f0 + TILE_F] = arr[
                :, base + NJ * FD : base + NJ * TILE_F
            ].reshape(128, NJ, fg)
        full[c * BC : (c + 1) * BC] = out3.transpose(0, 2, 1).reshape(BC, NJ, NJ)
    if run_kwargs:
        return full, res
    return full



# revision 2
# speedup vs baseline: 1.3637x; 1.3637x over previous
"""Trainium2 Bass kernel for nn_HarmonicOscillatorOrbitals.

out[b, i, j] = exp(-s^2/2) * H_j(s), s = omega * x[b, i, 0], j = 0..31
(physicists' Hermite polynomials), data-parallel over 8 NeuronCores on
the leading batch axis.

Per core (8192 batches = 262144 scalars as [128 partitions, E=2048]):
  t   = 2*omega*x                      (ACT)
  env = exp(-s^2/2)                    (ACT Square + Exp)
  G_0 = env, G_1 = t*env, G_k = t*G_{k-1} - 2(k-1)*G_{k-2}  (= env*H_k)

Engine assignment (measured on HW: DVE+GPSIMD contend on shared SBUF
ports and strictly serialize — GPSIMD is net-negative next to a busy
DVE; DVE+ACT coexist at full speed):
  DVE : whole f32 ladder, 2 ops/step over the full [128, 2048] row
        (tensor_mul q = t*G_{k-1}; scalar_tensor_tensor
         G_k = (G_{k-2} * -2(k-1)) + q)
  ACT : envelope, t, and every output cast: fp16 out with a per-order
        power-of-2 scale 2^(7-2k) folded into the activation-copy so
        the full f32 precision of the ladder survives the 16-bit store
        (fp16 ulp 2^-11 at a per-order normalized scale; simulated
        end-to-end rel err 3.3e-4 vs the 2e-2 gate).
  DMA : fp16 output = 16 MB/core instead of 33.5 MB f32.

Host: descale by 2^(2k-7) while unsharding (exact power-of-2).
"""

from contextlib import ExitStack

import numpy as np

import concourse.bacc as bacc
import concourse.mybir as mybir
import concourse.tile as tile
from concourse.bass_utils import run_bass_kernel_spmd

F32 = mybir.dt.float32
F16 = mybir.dt.float16
AF = mybir.ActivationFunctionType
ALU = mybir.AluOpType

NJ = 32          # number of Hermite orders
N_CORES = 8
B = 65536        # full batch
BC = B // N_CORES
E = BC * NJ // 128   # 2048 free elems per partition per core

A_EXP = 7        # G'_k = psi_k * 2^(A_EXP - 2k); max |G'_31| ~ 48.4k < fp16 max


def _scale(k):
    return float(2.0 ** (A_EXP - 2 * k))


def _build(e=E):
    nc = bacc.Bacc("TRN2", target_bir_lowering=False, debug=False)
    x_d = nc.dram_tensor("x", [128, e], F32, kind="ExternalInput").ap()
    om_d = nc.dram_tensor("om", [1, 1], F32, kind="ExternalInput").ap()
    # k-major fp16 dump: [128, NJ, e] flattened; host permutes + descales
    out_d = nc.dram_tensor("out", [128, NJ * e], F16, kind="ExternalOutput").ap()

    with tile.TileContext(nc) as tc, ExitStack() as ctx:
        cpool = ctx.enter_context(tc.tile_pool(name="const", bufs=1))
        gp = ctx.enter_context(tc.tile_pool(name="gp", bufs=6))
        qp = ctx.enter_context(tc.tile_pool(name="qp", bufs=2))
        sp = ctx.enter_context(tc.tile_pool(name="sp", bufs=2))

        om1 = cpool.tile([128, 1], F32)
        nc.sync.dma_start(om1[0:1, :], om_d[:, :])
        om2 = cpool.tile([128, 1], F32)
        nc.gpsimd.partition_broadcast(om2[:, :], om1[0:1, :])
        nc.scalar.mul(om2[:, :], om2[:, :], 2.0)  # om2 = 2*omega

        x_t = cpool.tile([128, e], F32)
        nc.sync.dma_start(x_t[:, :], x_d[:, :])
        t_t = cpool.tile([128, e], F32)
        nc.scalar.mul(t_t[:, :], x_t[:, :], om2[:, 0:1])  # t = 2*omega*x

        sq = cpool.tile([128, e], F32)
        nc.scalar.activation(sq[:, :], t_t[:, :], AF.Square, scale=0.5)  # s^2

        g = {}
        g[0] = gp.tile([128, e], F32, name="g0", tag="g")
        nc.scalar.activation(g[0][:, :], sq[:, :], AF.Exp, scale=-0.5)  # env
        g[1] = gp.tile([128, e], F32, name="g1", tag="g")
        nc.vector.tensor_mul(g[1][:, :], t_t[:, :], g[0][:, :])

        stage = {}

        def cast(k):
            qtr = k // 8
            if qtr not in stage:
                stage[qtr] = sp.tile([128, 8 * e], F16, name=f"st{qtr}", tag="st")
            off = (k % 8) * e
            nc.scalar.mul(stage[qtr][:, off : off + e], g[k][:, :], _scale(k))
            if k % 8 == 7:
                nc.sync.dma_start(
                    out_d[:, (k - 7) * e : (k + 1) * e], stage[qtr][:, :]
                )

        cast(0)
        cast(1)
        for k in range(2, NJ):
            c = 2.0 * (k - 1)
            q_t = qp.tile([128, e], F32, tag="q")
            nc.vector.tensor_mul(q_t[:, :], t_t[:, :], g[k - 1][:, :])
            g[k] = gp.tile([128, e], F32, name=f"g{k}", tag="g")
            nc.vector.scalar_tensor_tensor(
                g[k][:, :], g[k - 2][:, :], -c, q_t[:, :], ALU.mult, ALU.add
            )
            cast(k)
            g.pop(k - 2)

    nc.compile()
    return nc


_CACHED_NC = None

# host descale: psi_k = stored_k * 2^(2k - A_EXP)
_DESCALE = (2.0 ** (2.0 * np.arange(NJ) - A_EXP)).astype(np.float32)


def kernel(x: np.ndarray, omega_kernel: np.ndarray, **run_kwargs) -> np.ndarray:
    global _CACHED_NC
    assert x.shape == (B, NJ, 1) and omega_kernel.shape == (1, 1), (
        x.shape,
        omega_kernel.shape,
    )
    x = np.ascontiguousarray(x, np.float32)
    om = np.ascontiguousarray(omega_kernel, np.float32)

    if _CACHED_NC is None:
        _CACHED_NC = _build()
    nc = _CACHED_NC

    in_maps = [
        {
            "x": x[c * BC : (c + 1) * BC].reshape(128, E),
            "om": om,
        }
        for c in range(N_CORES)
    ]
    res = run_bass_kernel_spmd(nc, in_maps, core_ids=list(range(N_CORES)), **run_kwargs)
    full = np.empty((B, NJ, NJ), np.float32)
    for c in range(N_CORES):
        arr = np.asarray(res.results[c]["out"]).view(np.float16).reshape(128, NJ, E)
        out3 = arr.astype(np.float32)
        out3 *= _DESCALE[None, :, None]
        full[c * BC : (c + 1) * BC] = out3.transpose(0, 2, 1).reshape(BC, NJ, NJ)
    if run_kwargs:
        return full, res
    return full


# revision 3
# speedup vs baseline: 1.4987x; 1.0990x over previous
"""Trainium2 Bass kernel for nn_HarmonicOscillatorOrbitals.

out[b, i, j] = exp(-s^2/2) * H_j(s), s = omega * x[b, i, 0], j = 0..31
(physicists' Hermite polynomials), data-parallel over 8 NeuronCores on
the leading batch axis.

Per core (8192 batches = 262144 scalars as [128 partitions, E=2048]):
  env = exp(-(omega*x)^2/2), t = 2*omega*x
  G_0 = env, G_1 = t*env, G_k = t*G_{k-1} - 2(k-1)*G_{k-2}  (= env*H_k)

Engine assignment (measured on HW: DVE+GPSIMD contend on shared SBUF
ports and strictly serialize — GPSIMD is net-negative next to a busy
DVE; DVE+ACT coexist at full speed):
  DVE : whole f32 ladder, 2 ops/step (tensor_mul q = t*G_{k-1};
        scalar_tensor_tensor G_k = (G_{k-2} * -2(k-1)) + q), split in
        two column blocks so block A starts while block B is seeding.
  ACT : envelope, t, and every output cast: fp16 out with a per-order
        power-of-2 scale 2^(7-2k) folded into the activation-copy so
        the full f32 precision of the ladder survives the 16-bit store
        (simulated end-to-end rel err 3.3e-4 vs the 2e-2 gate).
  DMA : fp16 output = 16 MB/core; flushed per (block, 2 orders) to
        keep the drain after the last ladder step short.

Host: descale by 2^(2k-7) while unsharding (exact power-of-2).
"""

from contextlib import ExitStack

import numpy as np

import concourse.bacc as bacc
import concourse.mybir as mybir
import concourse.tile as tile
from concourse.bass_utils import run_bass_kernel_spmd

F32 = mybir.dt.float32
F16 = mybir.dt.float16
AF = mybir.ActivationFunctionType
ALU = mybir.AluOpType

NJ = 32          # number of Hermite orders
N_CORES = 8
B = 65536        # full batch
BC = B // N_CORES
E = BC * NJ // 128   # 2048 free elems per partition per core
NB = 2           # column blocks
BW = E // NB
FLUSH = 2        # orders per DMA flush

A_EXP = 7        # stored_k = psi_k * 2^(A_EXP - 2k); max |stored_31| ~ 48.4k


def _scale(k):
    return float(2.0 ** (A_EXP - 2 * k))


def _build():
    nc = bacc.Bacc("TRN2", target_bir_lowering=False, debug=False)
    x_d = nc.dram_tensor("x", [128, E], F32, kind="ExternalInput").ap()
    om_d = nc.dram_tensor("om", [1, 1], F32, kind="ExternalInput").ap()
    # block-major fp16 dump: [128, NB, NJ, BW] flattened
    out_d = nc.dram_tensor("out", [128, NJ * E], F16, kind="ExternalOutput").ap()

    with tile.TileContext(nc) as tc, ExitStack() as ctx:
        cpool = ctx.enter_context(tc.tile_pool(name="const", bufs=1))
        gpools = [
            ctx.enter_context(tc.tile_pool(name=f"g{b}", bufs=6)) for b in range(NB)
        ]
        qpools = [
            ctx.enter_context(tc.tile_pool(name=f"q{b}", bufs=2)) for b in range(NB)
        ]
        spools = [
            ctx.enter_context(tc.tile_pool(name=f"s{b}", bufs=3)) for b in range(NB)
        ]

        # omega chain ([128,1] ops) runs in parallel with the x DMA + Square
        om1 = cpool.tile([128, 1], F32)
        nc.sync.dma_start(om1[0:1, :], om_d[:, :])
        om2 = cpool.tile([128, 1], F32)
        nc.gpsimd.partition_broadcast(om2[:, :], om1[0:1, :])
        omsq = cpool.tile([128, 1], F32)
        nc.scalar.activation(omsq[:, :], om2[:, :], AF.Square)      # omega^2
        omneg = cpool.tile([128, 1], F32)
        nc.scalar.mul(omneg[:, :], omsq[:, :], -0.5)                # -omega^2/2
        nc.scalar.mul(om2[:, :], om2[:, :], 2.0)                    # 2*omega

        x_t = cpool.tile([128, E], F32)
        t_t = cpool.tile([128, E], F32)
        sqx = cpool.tile([128, E], F32)

        g = [{} for _ in range(NB)]
        for b in range(NB):
            lo = b * BW
            nc.sync.dma_start(x_t[:, lo : lo + BW], x_d[:, lo : lo + BW])
            nc.scalar.activation(
                sqx[:, lo : lo + BW], x_t[:, lo : lo + BW], AF.Square
            )  # x^2 (no omega dependency)
            g[b][0] = gpools[b].tile([128, BW], F32, name=f"g0_{b}", tag=f"g{b}")
            nc.scalar.activation(
                g[b][0][:, :], sqx[:, lo : lo + BW], AF.Exp, scale=omneg[:, 0:1]
            )  # env = exp(-omega^2 x^2 / 2)
            nc.scalar.mul(t_t[:, lo : lo + BW], x_t[:, lo : lo + BW], om2[:, 0:1])
            g[b][1] = gpools[b].tile([128, BW], F32, name=f"g1_{b}", tag=f"g{b}")
            nc.vector.tensor_mul(g[b][1][:, :], t_t[:, lo : lo + BW], g[b][0][:, :])

        stage = [{} for _ in range(NB)]

        def cast(b, k):
            grp = k // FLUSH
            if grp not in stage[b]:
                stage[b][grp] = spools[b].tile(
                    [128, FLUSH * BW], F16, name=f"st{b}_{grp}", tag=f"st{b}"
                )
            off = (k % FLUSH) * BW
            nc.scalar.mul(stage[b][grp][:, off : off + BW], g[b][k][:, :], _scale(k))
            if k % FLUSH == FLUSH - 1:
                base = b * NJ * BW + (k - FLUSH + 1) * BW
                nc.sync.dma_start(
                    out_d[:, base : base + FLUSH * BW], stage[b][grp][:, :]
                )

        for b in range(NB):
            cast(b, 0)
            cast(b, 1)
        for k in range(2, NJ):
            c = 2.0 * (k - 1)
            for b in range(NB):
                lo = b * BW
                q_t = qpools[b].tile([128, BW], F32, tag=f"q{b}")
                nc.vector.tensor_mul(q_t[:, :], t_t[:, lo : lo + BW], g[b][k - 1][:, :])
                g[b][k] = gpools[b].tile([128, BW], F32, name=f"g{k}_{b}", tag=f"g{b}")
                nc.vector.scalar_tensor_tensor(
                    g[b][k][:, :], g[b][k - 2][:, :], -c, q_t[:, :], ALU.mult, ALU.add
                )
            for b in range(NB):
                cast(b, k)
                g[b].pop(k - 2)

    nc.compile()
    return nc


_CACHED_NC = None

# host descale: psi_k = stored_k * 2^(2k - A_EXP)
_DESCALE = (2.0 ** (2.0 * np.arange(NJ) - A_EXP)).astype(np.float32)


def kernel(x: np.ndarray, omega_kernel: np.ndarray, **run_kwargs) -> np.ndarray:
    global _CACHED_NC
    assert x.shape == (B, NJ, 1) and omega_kernel.shape == (1, 1), (
        x.shape,
        omega_kernel.shape,
    )
    x = np.ascontiguousarray(x, np.float32)
    om = np.ascontiguousarray(omega_kernel, np.float32)

    if _CACHED_NC is None:
        _CACHED_NC = _build()
    nc = _CACHED_NC

    in_maps = [
        {
            "x": x[c * BC : (c + 1) * BC].reshape(128, E),
            "om": om,
        }
        for c in range(N_CORES)
    ]
    res = run_bass_kernel_spmd(nc, in_maps, core_ids=list(range(N_CORES)), **run_kwargs)
    full = np.empty((B, NJ, NJ), np.float32)
    out3 = np.empty((128, NJ, E), np.float32)
    for c in range(N_CORES):
        arr = np.asarray(res.results[c]["out"]).view(np.float16)
        for b in range(NB):
            blk = arr[:, b * NJ * BW : (b + 1) * NJ * BW].reshape(128, NJ, BW)
            out3[:, :, b * BW : (b + 1) * BW] = blk
        out3 *= _DESCALE[None, :, None]
        full[c * BC : (c + 1) * BC] = out3.transpose(0, 2, 1).reshape(BC, NJ, NJ)
    if run_kwargs:
        return full, res
    return full


# revision 4
# speedup vs baseline: 1.5412x; 1.0283x over previous
"""Trainium2 Bass kernel for nn_HarmonicOscillatorOrbitals.

out[b, i, j] = exp(-s^2/2) * H_j(s), s = omega * x[b, i, 0], j = 0..31
(physicists' Hermite polynomials), data-parallel over 8 NeuronCores on
the leading batch axis.

Per core (8192 batches = 262144 scalars as [128 partitions, E=2048]):
  env = exp(-(omega*x)^2/2), t = 2*omega*x
  G_0 = env, G_1 = t*env, G_k = t*G_{k-1} - 2(k-1)*G_{k-2}  (= env*H_k)

Engine assignment (measured on HW: DVE+GPSIMD contend on shared SBUF
ports and strictly serialize — GPSIMD is net-negative next to a busy
DVE; DVE+ACT coexist at full speed):
  DVE : whole f32 ladder, 2 ops/step (tensor_mul q = t*G_{k-1};
        scalar_tensor_tensor G_k = (G_{k-2} * -2(k-1)) + q), split in
        two column blocks; order 31 is folded straight into the fp16
        stage by the DVE (prescaled r via 2x tensor_scalar, then a
        scalar_tensor_tensor with fp16 output) so the epilogue never
        waits on an ACT cast.
  ACT : envelope, t, and the output casts: fp16 out with a per-order
        power-of-2 scale 2^(7-2k) folded into the activation-copy so
        the full f32 precision of the ladder survives the 16-bit store
        (simulated end-to-end rel err 3.3e-4 vs the 2e-2 gate).
  DMA : fp16 output = 16 MB/core, flushed per (block, 2 orders); x is
        loaded in 128 KB chunks so the first block seeds early. omega
        arrives host-replicated as [128,1] (no on-chip broadcast).

Host: descale by 2^(2k-7) while unsharding (exact power-of-2).
"""

from contextlib import ExitStack

import numpy as np

import concourse.bacc as bacc
import concourse.mybir as mybir
import concourse.tile as tile
from concourse.bass_utils import run_bass_kernel_spmd

F32 = mybir.dt.float32
F16 = mybir.dt.float16
AF = mybir.ActivationFunctionType
ALU = mybir.AluOpType

NJ = 32          # number of Hermite orders
N_CORES = 8
B = 65536        # full batch
BC = B // N_CORES
E = BC * NJ // 128   # 2048 free elems per partition per core
NB = 2           # column blocks
BW = E // NB
SEED_W = 512     # seed (Square/Exp/t) granularity
XCH = 4          # x-DMA chunks per block
FLUSH = 2        # orders per DMA flush

A_EXP = 7        # stored_k = psi_k * 2^(A_EXP - 2k); max |stored_31| ~ 48.4k


def _scale(k):
    return float(2.0 ** (A_EXP - 2 * k))


def _build():
    nc = bacc.Bacc("TRN2", target_bir_lowering=False, debug=False)
    x_d = nc.dram_tensor("x", [128, E], F32, kind="ExternalInput").ap()
    om_d = nc.dram_tensor("om", [128, 1], F32, kind="ExternalInput").ap()
    # block-major fp16 dump: [128, NB, NJ, BW] flattened
    out_d = nc.dram_tensor("out", [128, NJ * E], F16, kind="ExternalOutput").ap()

    with tile.TileContext(nc) as tc, ExitStack() as ctx:
        cpool = ctx.enter_context(tc.tile_pool(name="const", bufs=1))
        gpools = [
            ctx.enter_context(tc.tile_pool(name=f"g{b}", bufs=6)) for b in range(NB)
        ]
        qpools = [
            ctx.enter_context(tc.tile_pool(name=f"q{b}", bufs=2)) for b in range(NB)
        ]
        spools = [
            ctx.enter_context(tc.tile_pool(name=f"s{b}", bufs=3)) for b in range(NB)
        ]

        om2 = cpool.tile([128, 1], F32)
        nc.sync.dma_start(om2[:, :], om_d[:, :])
        omneg = cpool.tile([128, 1], F32)
        nc.scalar.activation(omneg[:, :], om2[:, :], AF.Square)  # omega^2
        nc.scalar.mul(omneg[:, :], omneg[:, :], -0.5)            # -omega^2/2
        nc.scalar.mul(om2[:, :], om2[:, :], 2.0)                 # 2*omega

        x_t = cpool.tile([128, E], F32)
        t_t = cpool.tile([128, E], F32)
        sqx = cpool.tile([128, E], F32)

        g = [{} for _ in range(NB)]
        for b in range(NB):
            lo = b * BW
            for ch in range(XCH):
                w = BW // XCH
                o = lo + ch * w
                nc.sync.dma_start(x_t[:, o : o + w], x_d[:, o : o + w])
            g[b][0] = gpools[b].tile([128, BW], F32, name=f"g0_{b}", tag=f"g{b}")
            for o in range(lo, lo + BW, SEED_W):
                sl = slice(o, o + SEED_W)
                nc.scalar.activation(sqx[:, sl], x_t[:, sl], AF.Square)
                nc.scalar.activation(
                    g[b][0][:, o - lo : o - lo + SEED_W],
                    sqx[:, sl],
                    AF.Exp,
                    scale=omneg[:, 0:1],
                )  # env = exp(-omega^2 x^2 / 2)
                nc.scalar.mul(t_t[:, sl], x_t[:, sl], om2[:, 0:1])
            g[b][1] = gpools[b].tile([128, BW], F32, name=f"g1_{b}", tag=f"g{b}")
            nc.vector.tensor_mul(g[b][1][:, :], t_t[:, lo : lo + BW], g[b][0][:, :])

        stage = [{} for _ in range(NB)]

        def stage_slot(b, k):
            grp = k // FLUSH
            if grp not in stage[b]:
                stage[b][grp] = spools[b].tile(
                    [128, FLUSH * BW], F16, name=f"st{b}_{grp}", tag=f"st{b}"
                )
            off = (k % FLUSH) * BW
            return stage[b][grp][:, off : off + BW]

        def flush(b, k):
            if k % FLUSH == FLUSH - 1:
                base = b * NJ * BW + (k - FLUSH + 1) * BW
                nc.sync.dma_start(
                    out_d[:, base : base + FLUSH * BW], stage[b][k // FLUSH][:, :]
                )

        def cast(b, k):
            nc.scalar.mul(stage_slot(b, k), g[b][k][:, :], _scale(k))
            flush(b, k)

        for b in range(NB):
            cast(b, 0)
            cast(b, 1)
        for k in range(2, NJ - 1):
            c = 2.0 * (k - 1)
            for b in range(NB):
                lo = b * BW
                q_t = qpools[b].tile([128, BW], F32, tag=f"q{b}")
                nc.vector.tensor_mul(q_t[:, :], t_t[:, lo : lo + BW], g[b][k - 1][:, :])
                g[b][k] = gpools[b].tile([128, BW], F32, name=f"g{k}_{b}", tag=f"g{b}")
                nc.vector.scalar_tensor_tensor(
                    g[b][k][:, :], g[b][k - 2][:, :], -c, q_t[:, :], ALU.mult, ALU.add
                )
            for b in range(NB):
                cast(b, k)
                g[b].pop(k - 2)

        # k = 31: DVE folds the scaled result straight into the fp16 stage:
        #   r = G_29 * (-c*s31)   (2x tensor_scalar, f32)
        #   stage = (q * s31) + r (scalar_tensor_tensor, fp16 out)
        k = NJ - 1
        c, s31 = 2.0 * (k - 1), _scale(NJ - 1)
        for b in range(NB):
            lo = b * BW
            q_t = qpools[b].tile([128, BW], F32, tag=f"q{b}")
            nc.vector.tensor_mul(q_t[:, :], t_t[:, lo : lo + BW], g[b][k - 1][:, :])
            r_t = qpools[b].tile([128, BW], F32, tag=f"q{b}")
            nc.vector.tensor_scalar_mul(r_t[:, :], g[b][k - 2][:, :], -c * s31)
            nc.vector.scalar_tensor_tensor(
                stage_slot(b, k), q_t[:, :], s31, r_t[:, :], ALU.mult, ALU.add
            )
            flush(b, k)

    nc.compile()
    return nc


_CACHED_NC = None

# host descale: psi_k = stored_k * 2^(2k - A_EXP)
_DESCALE = (2.0 ** (2.0 * np.arange(NJ) - A_EXP)).astype(np.float32)


def kernel(x: np.ndarray, omega_kernel: np.ndarray, **run_kwargs) -> np.ndarray:
    global _CACHED_NC
    assert x.shape == (B, NJ, 1) and omega_kernel.shape == (1, 1), (
        x.shape,
        omega_kernel.shape,
    )
    x = np.ascontiguousarray(x, np.float32)
    om = np.ascontiguousarray(
        np.broadcast_to(omega_kernel.astype(np.float32), (128, 1))
    )

    if _CACHED_NC is None:
        _CACHED_NC = _build()
    nc = _CACHED_NC

    in_maps = [
        {
            "x": x[c * BC : (c + 1) * BC].reshape(128, E),
            "om": om,
        }
        for c in range(N_CORES)
    ]
    res = run_bass_kernel_spmd(nc, in_maps, core_ids=list(range(N_CORES)), **run_kwargs)
    full = np.empty((B, NJ, NJ), np.float32)
    out3 = np.empty((128, NJ, E), np.float32)
    for c in range(N_CORES):
        arr = np.asarray(res.results[c]["out"]).view(np.float16)
        for b in range(NB):
            blk = arr[:, b * NJ * BW : (b + 1) * NJ * BW].reshape(128, NJ, BW)
            out3[:, :, b * BW : (b + 1) * BW] = blk
        out3 *= _DESCALE[None, :, None]
        full[c * BC : (c + 1) * BC] = out3.transpose(0, 2, 1).reshape(BC, NJ, NJ)
    if run_kwargs:
        return full, res
    return full


# revision 7
# speedup vs baseline: 1.5442x; 1.0020x over previous
"""Trainium2 Bass kernel for nn_HarmonicOscillatorOrbitals.

out[b, i, j] = exp(-s^2/2) * H_j(s), s = omega * x[b, i, 0], j = 0..31
(physicists' Hermite polynomials), data-parallel over 8 NeuronCores on
the leading batch axis.

Per core (8192 batches = 262144 scalars as [128 partitions, E=2048]):
  env = exp(-(omega*x)^2/2), t = 2*omega*x
  G_0 = env, G_1 = t*env, G_k = t*G_{k-1} - 2(k-1)*G_{k-2}  (= env*H_k)

Engine assignment (measured on HW: DVE+GPSIMD contend on shared SBUF
ports and strictly serialize — GPSIMD is net-negative next to a busy
DVE; DVE+ACT coexist at full speed):
  DVE : whole f32 ladder, 2 ops/step (tensor_mul q = t*G_{k-1};
        scalar_tensor_tensor G_k = (G_{k-2} * -2(k-1)) + q), split in
        two column blocks; order 31 is folded straight into the fp16
        stage by the DVE (prescaled r via 2x tensor_scalar, then a
        scalar_tensor_tensor with fp16 output) so the epilogue never
        waits on an ACT cast.
  ACT : envelope, t, and the output casts: fp16 out with a per-order
        power-of-2 scale 2^(7-2k) folded into the activation-copy so
        the full f32 precision of the ladder survives the 16-bit store
        (simulated end-to-end rel err 3.3e-4 vs the 2e-2 gate).
  DMA : fp16 output = 16 MB/core, flushed per (block, 2 orders); x is
        loaded in 128 KB chunks so the first block seeds early. omega
        arrives host-replicated as [128,1] (no on-chip broadcast).

Host: descale by 2^(2k-7) while unsharding (exact power-of-2).
"""

from contextlib import ExitStack

import numpy as np

import concourse.bacc as bacc
import concourse.mybir as mybir
import concourse.tile as tile
from concourse.bass_utils import run_bass_kernel_spmd

F32 = mybir.dt.float32
F16 = mybir.dt.float16
AF = mybir.ActivationFunctionType
ALU = mybir.AluOpType

NJ = 32          # number of Hermite orders
N_CORES = 8
B = 65536        # full batch
BC = B // N_CORES
E = BC * NJ // 128   # 2048 free elems per partition per core
NB = 2           # column blocks
BW = E // NB
SEED_W = 512     # seed (Square/Exp/t) granularity
XCH = 4          # x-DMA chunks per block
FLUSH = 2        # orders per DMA flush

A_EXP = 7        # stored_k = psi_k * 2^(A_EXP - 2k); max |stored_31| ~ 48.4k


def _scale(k):
    return float(2.0 ** (A_EXP - 2 * k))


def _build():
    nc = bacc.Bacc("TRN2", target_bir_lowering=False, debug=False)
    x_d = nc.dram_tensor("x", [128, E], F32, kind="ExternalInput").ap()
    om_d = nc.dram_tensor("om", [128, 1], F32, kind="ExternalInput").ap()
    # block-major fp16 dump: [128, NB, NJ, BW] flattened
    out_d = nc.dram_tensor("out", [128, NJ * E], F16, kind="ExternalOutput").ap()

    with tile.TileContext(nc) as tc, ExitStack() as ctx:
        cpool = ctx.enter_context(tc.tile_pool(name="const", bufs=1))
        gpools = [
            ctx.enter_context(tc.tile_pool(name=f"g{b}", bufs=6)) for b in range(NB)
        ]
        qpools = [
            ctx.enter_context(tc.tile_pool(name=f"q{b}", bufs=2)) for b in range(NB)
        ]
        spools = [
            ctx.enter_context(tc.tile_pool(name=f"s{b}", bufs=3)) for b in range(NB)
        ]

        om2 = cpool.tile([128, 1], F32)
        nc.sync.dma_start(om2[:, :], om_d[:, :])
        omneg = cpool.tile([128, 1], F32)
        nc.scalar.activation(omneg[:, :], om2[:, :], AF.Square)  # omega^2
        nc.scalar.mul(omneg[:, :], omneg[:, :], -0.5)            # -omega^2/2
        nc.scalar.mul(om2[:, :], om2[:, :], 2.0)                 # 2*omega

        x_t = cpool.tile([128, E], F32)
        t_t = cpool.tile([128, E], F32)
        sqx = cpool.tile([128, E], F32)

        g = [{} for _ in range(NB)]
        for b in range(NB):
            lo = b * BW
            for ch in range(XCH):
                w = BW // XCH
                o = lo + ch * w
                # alternate trigger engines -> different DGE rings, so the
                # chunks stream in parallel instead of queueing on one ring
                eng = nc.sync if ch % 2 == 0 else nc.scalar
                eng.dma_start(x_t[:, o : o + w], x_d[:, o : o + w])
            g[b][0] = gpools[b].tile([128, BW], F32, name=f"g0_{b}", tag=f"g{b}")
            for o in range(lo, lo + BW, SEED_W):
                sl = slice(o, o + SEED_W)
                nc.scalar.activation(sqx[:, sl], x_t[:, sl], AF.Square)
                nc.scalar.activation(
                    g[b][0][:, o - lo : o - lo + SEED_W],
                    sqx[:, sl],
                    AF.Exp,
                    scale=omneg[:, 0:1],
                )  # env = exp(-omega^2 x^2 / 2)
                nc.scalar.mul(t_t[:, sl], x_t[:, sl], om2[:, 0:1])
            g[b][1] = gpools[b].tile([128, BW], F32, name=f"g1_{b}", tag=f"g{b}")
            nc.vector.tensor_mul(g[b][1][:, :], t_t[:, lo : lo + BW], g[b][0][:, :])

        stage = [{} for _ in range(NB)]

        def stage_slot(b, k):
            grp = k // FLUSH
            if grp not in stage[b]:
                stage[b][grp] = spools[b].tile(
                    [128, FLUSH * BW], F16, name=f"st{b}_{grp}", tag=f"st{b}"
                )
            off = (k % FLUSH) * BW
            return stage[b][grp][:, off : off + BW]

        def flush(b, k):
            if k % FLUSH == FLUSH - 1:
                base = b * NJ * BW + (k - FLUSH + 1) * BW
                eng = nc.sync if b == 0 else nc.scalar
                eng.dma_start(
                    out_d[:, base : base + FLUSH * BW], stage[b][k // FLUSH][:, :]
                )

        def cast(b, k):
            nc.scalar.mul(stage_slot(b, k), g[b][k][:, :], _scale(k))
            flush(b, k)

        for b in range(NB):
            cast(b, 0)
            cast(b, 1)
        for k in range(2, NJ - 1):
            c = 2.0 * (k - 1)
            for b in range(NB):
                lo = b * BW
                q_t = qpools[b].tile([128, BW], F32, tag=f"q{b}")
                nc.vector.tensor_mul(q_t[:, :], t_t[:, lo : lo + BW], g[b][k - 1][:, :])
                g[b][k] = gpools[b].tile([128, BW], F32, name=f"g{k}_{b}", tag=f"g{b}")
                nc.vector.scalar_tensor_tensor(
                    g[b][k][:, :], g[b][k - 2][:, :], -c, q_t[:, :], ALU.mult, ALU.add
                )
            for b in range(NB):
                cast(b, k)
                g[b].pop(k - 2)

        # k = 31: DVE folds the scaled result straight into the fp16 stage:
        #   r = G_29 * (-c*s31)   (2x tensor_scalar, f32)
        #   stage = (q * s31) + r (scalar_tensor_tensor, fp16 out)
        k = NJ - 1
        c, s31 = 2.0 * (k - 1), _scale(NJ - 1)
        for b in range(NB):
            lo = b * BW
            q_t = qpools[b].tile([128, BW], F32, tag=f"q{b}")
            nc.vector.tensor_mul(q_t[:, :], t_t[:, lo : lo + BW], g[b][k - 1][:, :])
            r_t = qpools[b].tile([128, BW], F32, tag=f"q{b}")
            nc.vector.tensor_scalar_mul(r_t[:, :], g[b][k - 2][:, :], -c * s31)
            nc.vector.scalar_tensor_tensor(
                stage_slot(b, k), q_t[:, :], s31, r_t[:, :], ALU.mult, ALU.add
            )
            flush(b, k)

    nc.compile()
    return nc


_CACHED_NC = None

# host descale: psi_k = stored_k * 2^(2k - A_EXP)
_DESCALE = (2.0 ** (2.0 * np.arange(NJ) - A_EXP)).astype(np.float32)


def kernel(x: np.ndarray, omega_kernel: np.ndarray, **run_kwargs) -> np.ndarray:
    global _CACHED_NC
    assert x.shape == (B, NJ, 1) and omega_kernel.shape == (1, 1), (
        x.shape,
        omega_kernel.shape,
    )
    x = np.ascontiguousarray(x, np.float32)
    om = np.ascontiguousarray(
        np.broadcast_to(omega_kernel.astype(np.float32), (128, 1))
    )

    if _CACHED_NC is None:
        _CACHED_NC = _build()
    nc = _CACHED_NC

    in_maps = [
        {
            "x": x[c * BC : (c + 1) * BC].reshape(128, E),
            "om": om,
        }
        for c in range(N_CORES)
    ]
    res = run_bass_kernel_spmd(nc, in_maps, core_ids=list(range(N_CORES)), **run_kwargs)
    full = np.empty((B, NJ, NJ), np.float32)
    out3 = np.empty((128, NJ, E), np.float32)
    for c in range(N_CORES):
        arr = np.asarray(res.results[c]["out"]).view(np.float16)
        for b in range(NB):
            blk = arr[:, b * NJ * BW : (b + 1) * NJ * BW].reshape(128, NJ, BW)
            out3[:, :, b * BW : (b + 1) * BW] = blk
        out3 *= _DESCALE[None, :, None]
        full[c * BC : (c + 1) * BC] = out3.transpose(0, 2, 1).reshape(BC, NJ, NJ)
    if run_kwargs:
        return full, res
    return full


# revision 10
# speedup vs baseline: 1.5541x; 1.0064x over previous
"""Trainium2 Bass kernel for nn_HarmonicOscillatorOrbitals.

out[b, i, j] = exp(-s^2/2) * H_j(s), s = omega * x[b, i, 0], j = 0..31
(physicists' Hermite polynomials), data-parallel over 8 NeuronCores on
the leading batch axis.

Per core (8192 batches = 262144 scalars as [128 partitions, E=2048]):
  env = exp(-(omega*x)^2/2), t = 2*omega*x
  G_0 = env, G_1 = t*env, G_k = t*G_{k-1} - 2(k-1)*G_{k-2}  (= env*H_k)

Engine assignment (measured on HW: DVE+GPSIMD contend on shared SBUF
ports and strictly serialize — GPSIMD is net-negative next to a busy
DVE; DVE+ACT coexist at full speed):
  DVE : whole f32 ladder, 2 ops/step (tensor_mul q = t*G_{k-1};
        scalar_tensor_tensor G_k = (G_{k-2} * -2(k-1)) + q), split in
        two column blocks; order 31 is folded straight into the fp16
        stage by the DVE (prescaled r via 2x tensor_scalar, then a
        scalar_tensor_tensor with fp16 output) so the epilogue never
        waits on an ACT cast.
  ACT : envelope, t, and the output casts: fp16 out with a per-order
        power-of-2 scale 2^(7-2k) folded into the activation-copy so
        the full f32 precision of the ladder survives the 16-bit store
        (simulated end-to-end rel err 3.3e-4 vs the 2e-2 gate).
  DMA : fp16 output = 16 MB/core, flushed per (block, 2 orders); x is
        loaded in 128 KB chunks so the first block seeds early. omega
        arrives host-replicated as [128,1] (no on-chip broadcast).

Host: descale by 2^(2k-7) while unsharding (exact power-of-2).
"""

from contextlib import ExitStack

import numpy as np

import concourse.bacc as bacc
import concourse.mybir as mybir
import concourse.tile as tile
from concourse.bass_utils import run_bass_kernel_spmd

F32 = mybir.dt.float32
F16 = mybir.dt.float16
AF = mybir.ActivationFunctionType
ALU = mybir.AluOpType

NJ = 32          # number of Hermite orders
N_CORES = 8
B = 65536        # full batch
BC = B // N_CORES
E = BC * NJ // 128   # 2048 free elems per partition per core
NB = 2           # column blocks
BW = E // NB
SEED_W = 512     # seed (Square/Exp/t) granularity
XCH = 4          # x-DMA chunks per block
FLUSH = 2        # orders per DMA flush

A_EXP = 7        # stored_k = psi_k * 2^(A_EXP - 2k); max |stored_31| ~ 48.4k


def _scale(k):
    return float(2.0 ** (A_EXP - 2 * k))


def _build():
    nc = bacc.Bacc("TRN2", target_bir_lowering=False, debug=False)
    x_d = nc.dram_tensor("x", [128, E], F32, kind="ExternalInput").ap()
    om_d = nc.dram_tensor("om", [128, 1], F32, kind="ExternalInput").ap()
    # block-major fp16 dump: [128, NB, NJ, BW] flattened
    out_d = nc.dram_tensor("out", [128, NJ * E], F16, kind="ExternalOutput").ap()

    with tile.TileContext(nc) as tc, ExitStack() as ctx:
        cpool = ctx.enter_context(tc.tile_pool(name="const", bufs=1))
        gpools = [
            ctx.enter_context(tc.tile_pool(name=f"g{b}", bufs=6)) for b in range(NB)
        ]
        qpools = [
            ctx.enter_context(tc.tile_pool(name=f"q{b}", bufs=2)) for b in range(NB)
        ]
        spools = [
            ctx.enter_context(tc.tile_pool(name=f"s{b}", bufs=3)) for b in range(NB)
        ]

        om2 = cpool.tile([128, 1], F32)
        nc.sync.dma_start(om2[:, :], om_d[:, :])
        omneg = cpool.tile([128, 1], F32)
        nc.scalar.activation(omneg[:, :], om2[:, :], AF.Square)  # omega^2
        nc.scalar.mul(omneg[:, :], omneg[:, :], -0.5)            # -omega^2/2
        nc.scalar.mul(om2[:, :], om2[:, :], 2.0)                 # 2*omega

        x_t = cpool.tile([128, E], F32)
        t_t = cpool.tile([128, E], F32)
        sqx = cpool.tile([128, E], F32)

        g = [{} for _ in range(NB)]
        for b in range(NB):
            lo = b * BW
            for ch in range(XCH):
                w = BW // XCH
                o = lo + ch * w
                # alternate trigger engines -> different DGE rings, so the
                # chunks stream in parallel instead of queueing on one ring
                eng = nc.sync if ch % 2 == 0 else nc.scalar
                eng.dma_start(x_t[:, o : o + w], x_d[:, o : o + w])
            g[b][0] = gpools[b].tile([128, BW], F32, name=f"g0_{b}", tag=f"g{b}")
            for o in range(lo, lo + BW, SEED_W):
                sl = slice(o, o + SEED_W)
                nc.scalar.activation(sqx[:, sl], x_t[:, sl], AF.Square)
                nc.scalar.activation(
                    g[b][0][:, o - lo : o - lo + SEED_W],
                    sqx[:, sl],
                    AF.Exp,
                    scale=omneg[:, 0:1],
                )  # env = exp(-omega^2 x^2 / 2)
                nc.scalar.mul(t_t[:, sl], x_t[:, sl], om2[:, 0:1])
            g[b][1] = gpools[b].tile([128, BW], F32, name=f"g1_{b}", tag=f"g{b}")
            for o in range(0, BW, SEED_W):
                nc.vector.tensor_mul(
                    g[b][1][:, o : o + SEED_W],
                    t_t[:, lo + o : lo + o + SEED_W],
                    g[b][0][:, o : o + SEED_W],
                )

        stage = [{} for _ in range(NB)]

        def stage_slot(b, k):
            grp = k // FLUSH
            if grp not in stage[b]:
                stage[b][grp] = spools[b].tile(
                    [128, FLUSH * BW], F16, name=f"st{b}_{grp}", tag=f"st{b}"
                )
            off = (k % FLUSH) * BW
            return stage[b][grp][:, off : off + BW]

        def flush(b, k):
            eng = nc.sync if b == 0 else nc.scalar
            if k >= NJ - FLUSH:
                # epilogue: flush per order so the drain after the last
                # ladder step is one small DMA, not a whole group
                off = (k % FLUSH) * BW
                base = b * NJ * BW + k * BW
                eng.dma_start(
                    out_d[:, base : base + BW],
                    stage[b][k // FLUSH][:, off : off + BW],
                )
            elif k % FLUSH == FLUSH - 1:
                base = b * NJ * BW + (k - FLUSH + 1) * BW
                eng.dma_start(
                    out_d[:, base : base + FLUSH * BW], stage[b][k // FLUSH][:, :]
                )

        def cast(b, k):
            nc.scalar.mul(stage_slot(b, k), g[b][k][:, :], _scale(k))
            flush(b, k)

        for b in range(NB):
            cast(b, 0)
            cast(b, 1)
        for k in range(2, NJ - 1):
            c = 2.0 * (k - 1)
            # slice the first ladder steps so the chain starts as soon as
            # the first SEED_W columns are seeded (x DMA still in flight)
            sw = SEED_W if k <= 3 else BW
            for b in range(NB):
                lo = b * BW
                q_t = qpools[b].tile([128, BW], F32, tag=f"q{b}")
                g[b][k] = gpools[b].tile([128, BW], F32, name=f"g{k}_{b}", tag=f"g{b}")
                for o in range(0, BW, sw):
                    nc.vector.tensor_mul(
                        q_t[:, o : o + sw],
                        t_t[:, lo + o : lo + o + sw],
                        g[b][k - 1][:, o : o + sw],
                    )
                    nc.vector.scalar_tensor_tensor(
                        g[b][k][:, o : o + sw],
                        g[b][k - 2][:, o : o + sw],
                        -c,
                        q_t[:, o : o + sw],
                        ALU.mult,
                        ALU.add,
                    )
            for b in range(NB):
                cast(b, k)
                g[b].pop(k - 2)

        # k = 31: DVE folds the scaled result straight into the fp16 stage:
        #   r = G_29 * (-c*s31)   (2x tensor_scalar, f32)
        #   stage = (q * s31) + r (scalar_tensor_tensor, fp16 out)
        k = NJ - 1
        c, s31 = 2.0 * (k - 1), _scale(NJ - 1)
        for b in range(NB):
            lo = b * BW
            q_t = qpools[b].tile([128, BW], F32, tag=f"q{b}")
            nc.vector.tensor_mul(q_t[:, :], t_t[:, lo : lo + BW], g[b][k - 1][:, :])
            r_t = qpools[b].tile([128, BW], F32, tag=f"q{b}")
            nc.vector.tensor_scalar_mul(r_t[:, :], g[b][k - 2][:, :], -c * s31)
            nc.vector.scalar_tensor_tensor(
                stage_slot(b, k), q_t[:, :], s31, r_t[:, :], ALU.mult, ALU.add
            )
            flush(b, k)

    nc.compile()
    return nc


_CACHED_NC = None

# host descale: psi_k = stored_k * 2^(2k - A_EXP)
_DESCALE = (2.0 ** (2.0 * np.arange(NJ) - A_EXP)).astype(np.float32)


def kernel(x: np.ndarray, omega_kernel: np.ndarray, **run_kwargs) -> np.ndarray:
    global _CACHED_NC
    assert x.shape == (B, NJ, 1) and omega_kernel.shape == (1, 1), (
        x.shape,
        omega_kernel.shape,
    )
    x = np.ascontiguousarray(x, np.float32)
    om = np.ascontiguousarray(
        np.broadcast_to(omega_kernel.astype(np.float32), (128, 1))
    )

    if _CACHED_NC is None:
        _CACHED_NC = _build()
    nc = _CACHED_NC

    in_maps = [
        {
            "x": x[c * BC : (c + 1) * BC].reshape(128, E),
            "om": om,
        }
        for c in range(N_CORES)
    ]
    res = run_bass_kernel_spmd(nc, in_maps, core_ids=list(range(N_CORES)), **run_kwargs)
    full = np.empty((B, NJ, NJ), np.float32)
    out3 = np.empty((128, NJ, E), np.float32)
    for c in range(N_CORES):
        arr = np.asarray(res.results[c]["out"]).view(np.float16)
        for b in range(NB):
            blk = arr[:, b * NJ * BW : (b + 1) * NJ * BW].reshape(128, NJ, BW)
            out3[:, :, b * BW : (b + 1) * BW] = blk
        out3 *= _DESCALE[None, :, None]
        full[c * BC : (c + 1) * BC] = out3.transpose(0, 2, 1).reshape(BC, NJ, NJ)
    if run_kwargs:
        return full, res
    return full
